# revision 23
# baseline (speedup 1.0000x reference)
"""Trainium2 Bass kernel: sparse 7x7x7 stride-1 max-pool over a 64^3 voxel grid
(MinkowskiEngine semantics) + per-point MLP (1x1 conv -> ReLU -> 1x1 conv ->
sigmoid) * feats.

Strategy (8 NeuronCores, SPMD, no collectives):
  - Shard the dense grid along z: core k owns z in [8k, 8k+8), 3-voxel halo
    each side (ZH=14), replicated halo build -> no cross-core exchange.
  - The HOST builds the dense per-core grid directly in the on-chip layout
    [x-plane][c%128 partition][c//128 half][z][y] bf16 (empty = -1e30), plus a
    contiguous center-slab copy (owned z only) for the final multiply. The
    device does zero scatter/gather work: each x-plane is one contiguous
    448KB + 256KB DMA with 3584B rows; output is one 256KB store per plane.
  - Separable windowed max (7 = ladder of gaps 1,2,3) on DVE in bf16 (2x
    perf mode). y innermost: z-ladder shifts are whole 64-elem rows ->
    flat contiguous APs; y-ladder offset-tuned; x combined by streaming
    (m2x/m4x/px over plane tiles). ~6.7us DVE per plane is the bottleneck
    (Vector ~93% busy).
  - Fused per-plane MLP on PE: h = relu(W1.T @ px) (R on partitions);
    y2^T = W2.T @ h computed TRANSPOSED (C-halves on partitions) so the
    sigmoid output is channel-major like the grid; flat dense multiply
    out_plane = grid_center * sig at pipeline lag 5 (never waits on ACT);
    out-DMA via gpsimd (SWDGE) to keep the ACT queue clean.
  - Host gathers the N sparse rows from the dense product slabs.

Measured on the 8-core axon TRN2 fleet: HW exec ~456us (baseline of the
previous session: 870-918us), rel err 6.4e-3 vs the fp32 reference.

Notes for future tuning (measured): gpsimd tensor ops and dma accum_op are
REJECTED by this backend at load; DVE strided WRITES are the expensive APs
(oz ~0.70 cyc/elem vs ~0.54 dense); batching planes into deeper APs and
splitting pools by parity both REGRESSED.
"""

from contextlib import ExitStack
from dataclasses import dataclass

import numpy as np

C = 256
R = 128
SENT = -1.0e30


@dataclass(frozen=True)
class Cfg:
    D: int = 64           # grid extent per axis
    ZS: int = 8           # owned z-planes per core
    NPTS: int = 100000    # total points
    ncores: int = 8
    p_bufs: int = 8

    @property
    def ZH(self):
        return self.ZS + 6

    @property
    def YP(self):
        return self.D + 8     # padded y extent (4 sentinel each side)

    @property
    def NX(self):
        return self.D

    @property
    def PLANE(self):
        return 2 * self.ZH * self.YP     # elems per partition per input plane

    @property
    def VOX2(self):
        return 2 * self.ZS * self.D      # elems per partition per output plane


FULL = Cfg()


def build_nc(cfg: Cfg):
    """Build the (SPMD, per-core-identical) Bass program."""
    import concourse.bacc as bacc
    import concourse.tile as tile
    from concourse import mybir

    AF = mybir.ActivationFunctionType
    f32 = mybir.dt.float32
    dts = mybir.dt.bfloat16

    D, ZS, ZH, NX, YP = cfg.D, cfg.ZS, cfg.ZH, cfg.NX, cfg.YP
    PLANE, VOX2 = cfg.PLANE, cfg.VOX2

    nc = bacc.Bacc("TRN2", target_bir_lowering=False, debug=False,
                   enable_asserts=False, num_devices=cfg.ncores)

    g = nc.dram_tensor("g", [NX * 128, PLANE], dts, kind="ExternalInput").ap()
    gc = nc.dram_tensor("gc", [NX * 128, VOX2], dts, kind="ExternalInput").ap()
    w1 = nc.dram_tensor("w1", [C, R], dts, kind="ExternalInput").ap()
    w2 = nc.dram_tensor("w2", [R, C], dts, kind="ExternalInput").ap()
    out = nc.dram_tensor("out", [NX * 128, VOX2], dts, kind="ExternalOutput").ap()


    with tile.TileContext(nc) as tc, ExitStack() as ctx:
        const = ctx.enter_context(tc.tile_pool(name="const", bufs=1))
        pp = ctx.enter_context(tc.tile_pool(name="pp", bufs=cfg.p_bufs))
        ztp = ctx.enter_context(tc.tile_pool(name="ztp", bufs=2))
        ytp = ctx.enter_context(tc.tile_pool(name="ytp", bufs=2))
        oyp = ctx.enter_context(tc.tile_pool(name="oyp", bufs=4))
        m2xp = ctx.enter_context(tc.tile_pool(name="m2xp", bufs=4))
        m4xp = ctx.enter_context(tc.tile_pool(name="m4xp", bufs=8))
        pxp = ctx.enter_context(tc.tile_pool(name="pxp", bufs=3))
        hpp = ctx.enter_context(tc.tile_pool(name="hpp", bufs=2, space="PSUM"))
        y2p = ctx.enter_context(tc.tile_pool(name="y2p", bufs=2, space="PSUM"))
        hsp = ctx.enter_context(tc.tile_pool(name="hsp", bufs=3))
        sgp = ctx.enter_context(tc.tile_pool(name="sgp", bufs=4))
        prp = ctx.enter_context(tc.tile_pool(name="prp", bufs=3))
        cenp = ctx.enter_context(tc.tile_pool(name="cenp", bufs=8))

        # ---- constants
        neg = const.tile([128, VOX2], dts)
        nc.gpsimd.memset(neg[:], SENT)
        w1sb = const.tile([128, 2 * R], dts)
        nc.sync.dma_start(
            w1sb[:].rearrange("p (h r) -> p h r", h=2),
            w1.rearrange("(h p) r -> p h r", p=128),
        )
        w2sb = const.tile([128, C], dts)
        nc.sync.dma_start(w2sb[:], w2)
        w1v = w1sb[:].rearrange("p (h r) -> p h r", h=2)

        ozp = ctx.enter_context(tc.tile_pool(name="ozp", bufs=2))

        P_t, oy_t, m2x_t, m4x_t, sg_t = {}, {}, {}, {}, {}

        for i in range(NX + 5):
            if i < NX:
                # ---- load plane i: [p, 2h, ZH, D] (y innermost)
                P = pp.tile([128, PLANE], dts, name="P")
                nc.sync.dma_start(P[:], g[i * 128:(i + 1) * 128, :])
                cen = cenp.tile([128, VOX2], dts, tag="cen")
                nc.scalar.dma_start(cen[:], gc[i * 128:(i + 1) * 128, :])
                P_t[i] = cen
                # ---- z-ladder: y-borders are sentinel IN the host grid, so
                # sentinels propagate and every op is flat contiguous,
                # including a dense oz write (no ypad buffer needed)
                Pv2 = P[:].rearrange("p (h f) -> p h f", h=2)
                m2z = ztp.tile([128, 2 * (ZH - 1) * YP], dts, tag="m2z")
                m2zv2 = m2z[:].rearrange("p (h f) -> p h f", h=2)
                nc.vector.tensor_max(
                    m2zv2, Pv2[:, :, 0:(ZH - 1) * YP], Pv2[:, :, YP:ZH * YP])
                m4z = ztp.tile([128, 2 * (ZH - 3) * YP], dts, tag="m4z")
                m4zv2 = m4z[:].rearrange("p (h f) -> p h f", h=2)
                nc.vector.tensor_max(
                    m4zv2, m2zv2[:, :, 0:(ZH - 3) * YP],
                    m2zv2[:, :, 2 * YP:(ZH - 1) * YP])
                ozd = ozp.tile([128, 2 * ZS * YP], dts, tag="oz")
                ozv2 = ozd[:].rearrange("p (h f) -> p h f", h=2)
                nc.vector.tensor_max(
                    ozv2, m4zv2[:, :, 0:ZS * YP],
                    m4zv2[:, :, 3 * YP:(ZH - 3) * YP])

                # ---- y-ladder over ozd rows, owned y at [4, 4+D); (h,z) merged
                # oy[y] = max yp[y+1 .. y+8); m2y stored left-shifted by 1
                ypm = ozd[:].rearrange("p (g y) -> p g y", g=2 * ZS)
                m2y = ytp.tile([128, 2 * ZS * (YP - 2)], dts, tag="m2y")
                m2ym = m2y[:].rearrange("p (g y) -> p g y", g=2 * ZS)
                nc.vector.tensor_max(
                    m2ym, ypm[:, :, 1:YP - 1], ypm[:, :, 2:YP])
                m4y = ytp.tile([128, 2 * ZS * (YP - 4)], dts, tag="m4y")
                m4ym = m4y[:].rearrange("p (g y) -> p g y", g=2 * ZS)
                nc.vector.tensor_max(
                    m4ym, m2ym[:, :, 0:YP - 4], m2ym[:, :, 2:YP - 2])
                oy = oyp.tile([128, VOX2], dts, tag="oy")
                oym = oy[:].rearrange("p (g y) -> p g y", g=2 * ZS)
                nc.vector.tensor_max(
                    oym, m4ym[:, :, 0:D], m4ym[:, :, 3:3 + D])
                oy_t[i] = oy
            else:
                oy_t[i] = neg

            # ---- x-chain (streamed)
            j = i - 1
            if j >= NX:
                m2x_t[j] = neg
            else:
                m2x = m2xp.tile([128, VOX2], dts, tag="m2x")
                nc.vector.tensor_max(m2x[:], oy_t.get(j, neg)[:], oy_t[j + 1][:])
                m2x_t[j] = m2x
            j = i - 3
            if j >= NX:
                m4x_t[j] = neg
            else:
                a, b = m2x_t.get(j, neg), m2x_t.get(j + 2, neg)
                if a is neg and b is neg:
                    m4x_t[j] = neg
                else:
                    m4x = m4xp.tile([128, VOX2], dts, tag="m4x")
                    nc.vector.tensor_max(m4x[:], a[:], b[:])
                    m4x_t[j] = m4x
            k = i - 3
            if 0 <= k < NX:
                px = pxp.tile([128, VOX2], dts, tag="px")
                nc.vector.tensor_max(
                    px[:], m4x_t.get(k - 3, neg)[:], m4x_t[k][:])
                pxv = px[:].rearrange("p (h v) -> p h v", h=2)

                # ---- MLP on plane k
                hp = hpp.tile([128, ZS * D], f32, space="PSUM")
                for h in (0, 1):
                    nc.tensor.matmul(
                        hp[:], w1v[:, h, :], pxv[:, h, :], start=(h == 0), stop=(h == 1)
                    )
                hs = hsp.tile([128, ZS * D], dts)
                nc.scalar.activation(hs[:], hp[:], AF.Relu)
                # y2^T: [c-half partitions, vox] so sigmoid output is c-major
                y2 = y2p.tile([128, VOX2], f32, space="PSUM")
                for h in (0, 1):
                    nc.tensor.matmul(
                        y2[:, h * ZS * D:(h + 1) * ZS * D],
                        w2sb[:, h * 128:(h + 1) * 128],
                        hs[:],
                        start=True,
                        stop=True,
                    )
                sg = sgp.tile([128, VOX2], dts)
                nc.scalar.activation(sg[:], y2[:], AF.Sigmoid)
                sg_t[k] = sg

            # ---- dense multiply with the contiguous center slab, plane
            # k2 = i-5 (sg is 2 iterations old -> DVE never waits on ACT;
            # flat aligned operands -> full 2x mode)
            k2 = i - 5
            if 0 <= k2 < NX:
                prod = prp.tile([128, VOX2], dts)
                nc.vector.tensor_mul(prod[:], sg_t.pop(k2)[:], P_t.pop(k2)[:])
                nc.gpsimd.dma_start(out[k2 * 128:(k2 + 1) * 128, :], prod[:])

    nc.compile()
    return nc


def host_prep(cfg: Cfg, feats, coords, W1, W2):
    """Build per-core dense grids in device layout. Returns in_maps."""
    import ml_dtypes

    bf16 = ml_dtypes.bfloat16
    D, ZS, ZH, NX = cfg.D, cfg.ZS, cfg.ZH, cfg.NX

    ix = coords[:, 0].astype(np.int64)
    iy = coords[:, 1].astype(np.int64)
    iz = coords[:, 2].astype(np.int64)

    fb = feats.astype(bf16)
    # rows in [p, h] order: channel c lives at partition c%128, half c//128
    rows = np.ascontiguousarray(fb.reshape(-1, 2, 128).transpose(0, 2, 1))
    w1h = np.ascontiguousarray(W1.astype(bf16))
    w2h = np.ascontiguousarray(W2.astype(bf16))

    in_maps = []
    for k in range(cfg.ncores):
        zlo = k * ZS - 3
        sel = (iz >= zlo) & (iz < zlo + ZH)
        G = np.full((NX, 128, 2, ZH, cfg.YP), SENT, bf16)
        G[ix[sel], :, :, iz[sel] - zlo, 4 + iy[sel]] = rows[sel]
        gc = np.ascontiguousarray(G[:, :, :, 3:3 + ZS, 4:4 + D])
        in_maps.append({
            "g": G.reshape(NX * 128, cfg.PLANE),
            "gc": gc.reshape(NX * 128, cfg.VOX2),
            "w1": w1h,
            "w2": w2h,
        })
    return in_maps


def host_gather(cfg: Cfg, results, coords):
    """Gather sparse rows from the dense per-core product slabs."""
    D, ZS, ZH, NX = cfg.D, cfg.ZS, cfg.ZH, cfg.NX
    ix = coords[:, 0].astype(np.int64)
    iy = coords[:, 1].astype(np.int64)
    iz = coords[:, 2].astype(np.int64)
    out_full = np.empty((coords.shape[0], C), np.float32)
    for k in range(cfg.ncores):
        own = (iz >= k * ZS) & (iz < (k + 1) * ZS)
        O = results[k]["out"].reshape(NX, 128, 2, ZS, D)
        vals = O[ix[own], :, :, iz[own] - k * ZS, iy[own]]  # [n, 128, 2]
        out_full[own] = (
            vals.transpose(0, 2, 1).reshape(-1, C).astype(np.float32)
        )
    return out_full


_CACHE = {}


def _get_nc(cfg: Cfg):
    if cfg not in _CACHE:
        _CACHE[cfg] = build_nc(cfg)
    return _CACHE[cfg]


def kernel(feats, coords, W1, W2):
    from concourse.bass_utils import run_bass_kernel_spmd

    cfg = FULL
    nc = _get_nc(cfg)
    coords = np.asarray(coords)
    in_maps = host_prep(
        cfg,
        np.asarray(feats, np.float32),
        coords,
        np.asarray(W1, np.float32),
        np.asarray(W2, np.float32),
    )
    res = run_bass_kernel_spmd(nc, in_maps, core_ids=list(range(cfg.ncores)))
    return host_gather(cfg, res.results, coords)


# revision 24
# speedup vs baseline: 1.2452x; 1.2452x over previous
"""Trainium2 Bass kernel: sparse 7x7x7 stride-1 max-pool over a 64^3 voxel grid
(MinkowskiEngine semantics) + per-point MLP (1x1 conv -> ReLU -> 1x1 conv ->
sigmoid) * feats.

Strategy (8 NeuronCores, SPMD, no collectives):
  - Shard the dense grid along z: core k owns z in [8k, 8k+8), 3-voxel halo
    each side (ZH=14), replicated halo build -> no cross-core exchange.
  - The HOST builds the dense per-core grid directly in the on-chip layout
    [x-plane][c%128 partition][c//128 half][z][y] bf16 (empty = -1e30), plus a
    contiguous center-slab copy (owned z only) for the final multiply. The
    device does zero scatter/gather work: each x-plane is one contiguous
    448KB + 256KB DMA with 3584B rows; output is one 256KB store per plane.
  - Separable windowed max (7 = ladder of gaps 1,2,3) on DVE in bf16 (2x
    perf mode). y innermost: z-ladder shifts are whole 64-elem rows ->
    flat contiguous APs; y-ladder offset-tuned; x combined by streaming
    (m2x/m4x/px over plane tiles). ~6.7us DVE per plane is the bottleneck
    (Vector ~93% busy).
  - Fused per-plane MLP on PE: h = relu(W1.T @ px) (R on partitions);
    y2^T = W2.T @ h computed TRANSPOSED (C-halves on partitions) so the
    sigmoid output is channel-major like the grid; flat dense multiply
    out_plane = grid_center * sig at pipeline lag 5 (never waits on ACT);
    out-DMA via gpsimd (SWDGE) to keep the ACT queue clean.
  - Host gathers the N sparse rows from the dense product slabs.

Measured on the 8-core axon TRN2 fleet: HW exec ~456us (baseline of the
previous session: 870-918us), rel err 6.4e-3 vs the fp32 reference.

Notes for future tuning (measured): gpsimd tensor ops and dma accum_op are
REJECTED by this backend at load; DVE strided WRITES are the expensive APs
(oz ~0.70 cyc/elem vs ~0.54 dense); batching planes into deeper APs and
splitting pools by parity both REGRESSED.
"""

from contextlib import ExitStack
from dataclasses import dataclass

import numpy as np

C = 256
R = 128
SENT = -1.0e30


@dataclass(frozen=True)
class Cfg:
    D: int = 64           # grid extent per axis
    ZS: int = 8           # owned z-planes per core
    NPTS: int = 100000    # total points
    ncores: int = 8
    p_bufs: int = 8

    @property
    def ZH(self):
        return self.ZS + 6

    @property
    def YP(self):
        return self.D + 8     # padded y extent (4 sentinel each side)

    @property
    def NX(self):
        return self.D

    @property
    def PLANE(self):
        return 2 * self.ZH * self.D      # elems per partition per input plane

    @property
    def VOX2(self):
        return 2 * self.ZS * self.D      # elems per partition per output plane


FULL = Cfg()


def build_nc(cfg: Cfg):
    """Build the (SPMD, per-core-identical) Bass program."""
    import concourse.bacc as bacc
    import concourse.tile as tile
    from concourse import mybir

    AF = mybir.ActivationFunctionType
    f32 = mybir.dt.float32
    dts = mybir.dt.bfloat16

    D, ZS, ZH, NX, YP = cfg.D, cfg.ZS, cfg.ZH, cfg.NX, cfg.YP
    PLANE, VOX2 = cfg.PLANE, cfg.VOX2

    nc = bacc.Bacc("TRN2", target_bir_lowering=False, debug=False,
                   enable_asserts=False, num_devices=cfg.ncores)

    g = nc.dram_tensor("g", [NX * 128, PLANE], dts, kind="ExternalInput").ap()
    gc = nc.dram_tensor("gc", [NX * 128, VOX2], dts, kind="ExternalInput").ap()
    w1 = nc.dram_tensor("w1", [C, R], dts, kind="ExternalInput").ap()
    w2 = nc.dram_tensor("w2", [R, C], dts, kind="ExternalInput").ap()
    out = nc.dram_tensor("out", [NX * 128, VOX2], dts, kind="ExternalOutput").ap()


    with tile.TileContext(nc) as tc, ExitStack() as ctx:
        const = ctx.enter_context(tc.tile_pool(name="const", bufs=1))
        pp = ctx.enter_context(tc.tile_pool(name="pp", bufs=cfg.p_bufs))
        ztp = ctx.enter_context(tc.tile_pool(name="ztp", bufs=2))
        ytp = ctx.enter_context(tc.tile_pool(name="ytp", bufs=2))
        oyp = ctx.enter_context(tc.tile_pool(name="oyp", bufs=4))
        m2xp = ctx.enter_context(tc.tile_pool(name="m2xp", bufs=4))
        m4xp = ctx.enter_context(tc.tile_pool(name="m4xp", bufs=8))
        pxp = ctx.enter_context(tc.tile_pool(name="pxp", bufs=3))
        hpp = ctx.enter_context(tc.tile_pool(name="hpp", bufs=2, space="PSUM"))
        y2p = ctx.enter_context(tc.tile_pool(name="y2p", bufs=2, space="PSUM"))
        hsp = ctx.enter_context(tc.tile_pool(name="hsp", bufs=3))
        sgp = ctx.enter_context(tc.tile_pool(name="sgp", bufs=4))
        prp = ctx.enter_context(tc.tile_pool(name="prp", bufs=3))
        cenp = ctx.enter_context(tc.tile_pool(name="cenp", bufs=8))

        # ---- constants
        neg = const.tile([128, VOX2], dts)
        nc.gpsimd.memset(neg[:], SENT)
        w1sb = const.tile([128, 2 * R], dts)
        nc.sync.dma_start(
            w1sb[:].rearrange("p (h r) -> p h r", h=2),
            w1.rearrange("(h p) r -> p h r", p=128),
        )
        w2sb = const.tile([128, C], dts)
        nc.sync.dma_start(w2sb[:], w2)
        w1v = w1sb[:].rearrange("p (h r) -> p h r", h=2)

        # persistent y-padded buffer [p, 2h, ZS, YP]; y borders sentinel
        ypad = const.tile([128, 2 * ZS * YP], dts)
        ypv = ypad[:].rearrange("p (h z y) -> p h z y", h=2, z=ZS)
        nc.gpsimd.memset(ypv[:, :, :, 0:4], SENT)
        nc.gpsimd.memset(ypv[:, :, :, D + 4:YP], SENT)

        P_t, oy_t, m2x_t, m4x_t, sg_t = {}, {}, {}, {}, {}

        for i in range(NX + 5):
            if i < NX:
                # ---- load plane i: [p, 2h, ZH, D] (y innermost)
                P = pp.tile([128, PLANE], dts, name="P")
                nc.sync.dma_start(P[:], g[i * 128:(i + 1) * 128, :])
                cen = cenp.tile([128, VOX2], dts, tag="cen")
                nc.scalar.dma_start(cen[:], gc[i * 128:(i + 1) * 128, :])
                P_t[i] = cen
                # ---- z-ladder: row shifts are whole 64-elem multiples, so
                # every operand is a CONTIGUOUS run -> flat [p, h, run] APs
                Pv2 = P[:].rearrange("p (h f) -> p h f", h=2)
                m2z = ztp.tile([128, 2 * (ZH - 1) * D], dts, tag="m2z")
                m2zv2 = m2z[:].rearrange("p (h f) -> p h f", h=2)
                nc.vector.tensor_max(
                    m2zv2, Pv2[:, :, 0:(ZH - 1) * D], Pv2[:, :, D:ZH * D])
                m4z = ztp.tile([128, 2 * (ZH - 3) * D], dts, tag="m4z")
                m4zv2 = m4z[:].rearrange("p (h f) -> p h f", h=2)
                nc.vector.tensor_max(
                    m4zv2, m2zv2[:, :, 0:(ZH - 3) * D],
                    m2zv2[:, :, 2 * D:(ZH - 1) * D])
                m4zv = m4z[:].rearrange("p (h z y) -> p h z y", h=2, z=ZH - 3)
                nc.vector.tensor_max(
                    ypv[:, :, :, 4:4 + D], m4zv[:, :, 0:ZS, :], m4zv[:, :, 3:3 + ZS, :])

                # ---- y-ladder over ypad, owned y at [4, 4+D); (h,z) merged
                # oy[y] = max ypad[y+1 .. y+8); m2y stored left-shifted by 1
                ypm = ypad[:].rearrange("p (g y) -> p g y", g=2 * ZS)
                m2y = ytp.tile([128, 2 * ZS * (YP - 2)], dts, tag="m2y")
                m2ym = m2y[:].rearrange("p (g y) -> p g y", g=2 * ZS)
                nc.vector.tensor_max(
                    m2ym, ypm[:, :, 1:YP - 1], ypm[:, :, 2:YP])
                m4y = ytp.tile([128, 2 * ZS * (YP - 4)], dts, tag="m4y")
                m4ym = m4y[:].rearrange("p (g y) -> p g y", g=2 * ZS)
                nc.vector.tensor_max(
                    m4ym, m2ym[:, :, 0:YP - 4], m2ym[:, :, 2:YP - 2])
                oy = oyp.tile([128, VOX2], dts, tag="oy")
                oym = oy[:].rearrange("p (g y) -> p g y", g=2 * ZS)
                nc.vector.tensor_max(
                    oym, m4ym[:, :, 0:D], m4ym[:, :, 3:3 + D])
                oy_t[i] = oy
            else:
                oy_t[i] = neg

            # ---- x-chain (streamed)
            j = i - 1
            if j >= NX:
                m2x_t[j] = neg
            else:
                m2x = m2xp.tile([128, VOX2], dts, tag="m2x")
                nc.vector.tensor_max(m2x[:], oy_t.get(j, neg)[:], oy_t[j + 1][:])
                m2x_t[j] = m2x
            j = i - 3
            if j >= NX:
                m4x_t[j] = neg
            else:
                a, b = m2x_t.get(j, neg), m2x_t.get(j + 2, neg)
                if a is neg and b is neg:
                    m4x_t[j] = neg
                else:
                    m4x = m4xp.tile([128, VOX2], dts, tag="m4x")
                    nc.vector.tensor_max(m4x[:], a[:], b[:])
                    m4x_t[j] = m4x
            k = i - 3
            if 0 <= k < NX:
                px = pxp.tile([128, VOX2], dts, tag="px")
                nc.vector.tensor_max(
                    px[:], m4x_t.get(k - 3, neg)[:], m4x_t[k][:])
                pxv = px[:].rearrange("p (h v) -> p h v", h=2)

                # ---- MLP on plane k
                hp = hpp.tile([128, ZS * D], f32, space="PSUM")
                for h in (0, 1):
                    nc.tensor.matmul(
                        hp[:], w1v[:, h, :], pxv[:, h, :], start=(h == 0), stop=(h == 1)
                    )
                hs = hsp.tile([128, ZS * D], dts)
                nc.scalar.activation(hs[:], hp[:], AF.Relu)
                # y2^T: [c-half partitions, vox] so sigmoid output is c-major
                y2 = y2p.tile([128, VOX2], f32, space="PSUM")
                for h in (0, 1):
                    nc.tensor.matmul(
                        y2[:, h * ZS * D:(h + 1) * ZS * D],
                        w2sb[:, h * 128:(h + 1) * 128],
                        hs[:],
                        start=True,
                        stop=True,
                    )
                sg = sgp.tile([128, VOX2], dts)
                nc.scalar.activation(sg[:], y2[:], AF.Sigmoid)
                sg_t[k] = sg

            # ---- dense multiply with the contiguous center slab, plane
            # k2 = i-5 (sg is 2 iterations old -> DVE never waits on ACT;
            # flat aligned operands -> full 2x mode)
            k2 = i - 5
            if 0 <= k2 < NX:
                prod = prp.tile([128, VOX2], dts)
                nc.vector.tensor_mul(prod[:], sg_t.pop(k2)[:], P_t.pop(k2)[:])
                nc.gpsimd.dma_start(out[k2 * 128:(k2 + 1) * 128, :], prod[:])

    nc.compile()
    return nc


def host_prep(cfg: Cfg, feats, coords, W1, W2):
    """Build per-core dense grids in device layout. Returns in_maps."""
    import ml_dtypes

    bf16 = ml_dtypes.bfloat16
    D, ZS, ZH, NX = cfg.D, cfg.ZS, cfg.ZH, cfg.NX

    ix = coords[:, 0].astype(np.int64)
    iy = coords[:, 1].astype(np.int64)
    iz = coords[:, 2].astype(np.int64)

    fb = feats.astype(bf16)
    # rows in [p, h] order: channel c lives at partition c%128, half c//128
    rows = np.ascontiguousarray(fb.reshape(-1, 2, 128).transpose(0, 2, 1))
    w1h = np.ascontiguousarray(W1.astype(bf16))
    w2h = np.ascontiguousarray(W2.astype(bf16))

    in_maps = []
    for k in range(cfg.ncores):
        zlo = k * ZS - 3
        sel = (iz >= zlo) & (iz < zlo + ZH)
        G = np.full((NX, 128, 2, ZH, D), SENT, bf16)
        G[ix[sel], :, :, iz[sel] - zlo, iy[sel]] = rows[sel]
        gc = np.ascontiguousarray(G[:, :, :, 3:3 + ZS, :])
        in_maps.append({
            "g": G.reshape(NX * 128, cfg.PLANE),
            "gc": gc.reshape(NX * 128, cfg.VOX2),
            "w1": w1h,
            "w2": w2h,
        })
    return in_maps


def host_gather(cfg: Cfg, results, coords):
    """Gather sparse rows from the dense per-core product slabs."""
    D, ZS, ZH, NX = cfg.D, cfg.ZS, cfg.ZH, cfg.NX
    ix = coords[:, 0].astype(np.int64)
    iy = coords[:, 1].astype(np.int64)
    iz = coords[:, 2].astype(np.int64)
    out_full = np.empty((coords.shape[0], C), np.float32)
    for k in range(cfg.ncores):
        own = (iz >= k * ZS) & (iz < (k + 1) * ZS)
        O = results[k]["out"].reshape(NX, 128, 2, ZS, D)
        vals = O[ix[own], :, :, iz[own] - k * ZS, iy[own]]  # [n, 128, 2]
        out_full[own] = (
            vals.transpose(0, 2, 1).reshape(-1, C).astype(np.float32)
        )
    return out_full


_CACHE = {}


def _get_nc(cfg: Cfg):
    if cfg not in _CACHE:
        _CACHE[cfg] = build_nc(cfg)
    return _CACHE[cfg]


def kernel(feats, coords, W1, W2):
    from concourse.bass_utils import run_bass_kernel_spmd

    cfg = FULL
    nc = _get_nc(cfg)
    coords = np.asarray(coords)
    in_maps = host_prep(
        cfg,
        np.asarray(feats, np.float32),
        coords,
        np.asarray(W1, np.float32),
        np.asarray(W2, np.float32),
    )
    res = run_bass_kernel_spmd(nc, in_maps, core_ids=list(range(cfg.ncores)))
    return host_gather(cfg, res.results, coords)


# revision 26
# speedup vs baseline: 1.2760x; 1.0248x over previous
"""Trainium2 Bass kernel: sparse 7x7x7 stride-1 max-pool over a 64^3 voxel grid
(MinkowskiEngine semantics) + per-point MLP (1x1 conv -> ReLU -> 1x1 conv ->
sigmoid) * feats.

Strategy (8 NeuronCores, SPMD, no collectives):
  - Shard the dense grid along z: core k owns z in [8k, 8k+8), 3-voxel halo
    each side (ZH=14), replicated halo build -> no cross-core exchange.
  - The HOST builds the dense per-core grid directly in the on-chip layout
    [x-plane][c%128 partition][c//128 half][z][y] bf16 (empty = -1e30), plus a
    contiguous center-slab copy (owned z only) for the final multiply. The
    device does zero scatter/gather work: each x-plane is one contiguous
    448KB + 256KB DMA with 3584B rows; output is one 256KB store per plane.
  - Separable windowed max (7 = ladder of gaps 1,2,3) on DVE in bf16 (2x
    perf mode). y innermost: z-ladder shifts are whole 64-elem rows ->
    flat contiguous APs; y-ladder offset-tuned; x combined by streaming
    (m2x/m4x/px over plane tiles). ~6.7us DVE per plane is the bottleneck
    (Vector ~93% busy).
  - Fused per-plane MLP on PE: h = relu(W1.T @ px) (R on partitions);
    y2^T = W2.T @ h computed TRANSPOSED (C-halves on partitions) so the
    sigmoid output is channel-major like the grid; flat dense multiply
    out_plane = grid_center * sig at pipeline lag 5 (never waits on ACT);
    out-DMA via gpsimd (SWDGE) to keep the ACT queue clean.
  - Host gathers the N sparse rows from the dense product slabs.

Measured on the 8-core axon TRN2 fleet: HW exec ~456us (baseline of the
previous session: 870-918us), rel err 6.4e-3 vs the fp32 reference.

Notes for future tuning (measured): gpsimd tensor ops and dma accum_op are
REJECTED by this backend at load; DVE strided WRITES are the expensive APs
(oz ~0.70 cyc/elem vs ~0.54 dense); batching planes into deeper APs and
splitting pools by parity both REGRESSED.
"""

from contextlib import ExitStack
from dataclasses import dataclass

import numpy as np

C = 256
R = 128
SENT = -1.0e30


@dataclass(frozen=True)
class Cfg:
    D: int = 64           # grid extent per axis
    ZS: int = 8           # owned z-planes per core
    NPTS: int = 100000    # total points
    ncores: int = 8
    p_bufs: int = 8

    @property
    def ZH(self):
        return self.ZS + 6

    @property
    def YP(self):
        return self.D + 8     # padded y extent (4 sentinel each side)

    @property
    def NX(self):
        return self.D

    @property
    def PLANE(self):
        return 2 * self.ZH * self.D      # elems per partition per input plane

    @property
    def VOX2(self):
        return 2 * self.ZS * self.D      # elems per partition per output plane


FULL = Cfg()


def build_nc(cfg: Cfg):
    """Build the (SPMD, per-core-identical) Bass program."""
    import concourse.bacc as bacc
    import concourse.tile as tile
    from concourse import mybir

    AF = mybir.ActivationFunctionType
    f32 = mybir.dt.float32
    dts = mybir.dt.bfloat16

    D, ZS, ZH, NX, YP = cfg.D, cfg.ZS, cfg.ZH, cfg.NX, cfg.YP
    PLANE, VOX2 = cfg.PLANE, cfg.VOX2

    nc = bacc.Bacc("TRN2", target_bir_lowering=False, debug=False,
                   enable_asserts=False, num_devices=cfg.ncores)

    g = nc.dram_tensor("g", [NX * 128, PLANE], dts, kind="ExternalInput").ap()
    gc = nc.dram_tensor("gc", [NX * 128, VOX2], dts, kind="ExternalInput").ap()
    w1 = nc.dram_tensor("w1", [C, R], dts, kind="ExternalInput").ap()
    w2 = nc.dram_tensor("w2", [R, C], dts, kind="ExternalInput").ap()
    out = nc.dram_tensor("out", [NX * 128, VOX2], dts, kind="ExternalOutput").ap()


    with tile.TileContext(nc) as tc, ExitStack() as ctx:
        const = ctx.enter_context(tc.tile_pool(name="const", bufs=1))
        pp = ctx.enter_context(tc.tile_pool(name="pp", bufs=4))
        ztp = ctx.enter_context(tc.tile_pool(name="ztp", bufs=2))
        ytp = ctx.enter_context(tc.tile_pool(name="ytp", bufs=2))
        oyp = ctx.enter_context(tc.tile_pool(name="oyp", bufs=4))
        m2xp = ctx.enter_context(tc.tile_pool(name="m2xp", bufs=4))
        m4xp = ctx.enter_context(tc.tile_pool(name="m4xp", bufs=8))
        pxp = ctx.enter_context(tc.tile_pool(name="pxp", bufs=3))
        hpp = ctx.enter_context(tc.tile_pool(name="hpp", bufs=2, space="PSUM"))
        y2p = ctx.enter_context(tc.tile_pool(name="y2p", bufs=2, space="PSUM"))
        hsp = ctx.enter_context(tc.tile_pool(name="hsp", bufs=3))
        sgp = ctx.enter_context(tc.tile_pool(name="sgp", bufs=4))
        prp = ctx.enter_context(tc.tile_pool(name="prp", bufs=3))
        cenp = ctx.enter_context(tc.tile_pool(name="cenp", bufs=6))

        # ---- constants
        neg = const.tile([128, VOX2], dts)
        nc.gpsimd.memset(neg[:], SENT)
        w1sb = const.tile([128, 2 * R], dts)
        nc.sync.dma_start(
            w1sb[:].rearrange("p (h r) -> p h r", h=2),
            w1.rearrange("(h p) r -> p h r", p=128),
        )
        w2sb = const.tile([128, C], dts)
        nc.sync.dma_start(w2sb[:], w2)
        w1v = w1sb[:].rearrange("p (h r) -> p h r", h=2)

        Pc_t, P_t, oy_t, m2x_t, m4x_t, sg_t = {}, {}, {}, {}, {}, {}

        # batched y-padded buffer for 2 planes [p, (2x 2h ZS), YP]
        ypad = const.tile([128, 2 * 2 * ZS * YP], dts)
        ypg = ypad[:].rearrange("p (g y) -> p g y", g=4 * ZS)
        ypv = ypad[:].rearrange("p (g z y) -> p g z y", g=4, z=ZS)
        nc.gpsimd.memset(ypg[:, :, 0:4], SENT)
        nc.gpsimd.memset(ypg[:, :, D + 4:YP], SENT)

        def plane_tail(i):
            """x-chain, MLP (lag 3) and multiply+store (lag 5) for plane i."""
            j = i - 1
            if j >= NX:
                m2x_t[j] = neg
            else:
                m2x = m2xp.tile([128, VOX2], dts, tag="m2x")
                nc.vector.tensor_max(m2x[:], oy_t.get(j, neg[:]), oy_t[j + 1])
                m2x_t[j] = m2x
            j = i - 3
            if j >= NX:
                m4x_t[j] = neg
            else:
                a, b = m2x_t.get(j, neg), m2x_t.get(j + 2, neg)
                if a is neg and b is neg:
                    m4x_t[j] = neg
                else:
                    m4x = m4xp.tile([128, VOX2], dts, tag="m4x")
                    nc.vector.tensor_max(m4x[:], a[:], b[:])
                    m4x_t[j] = m4x
            k = i - 3
            if 0 <= k < NX:
                px = pxp.tile([128, VOX2], dts, tag="px")
                nc.vector.tensor_max(
                    px[:], m4x_t.get(k - 3, neg)[:], m4x_t[k][:])
                pxv = px[:].rearrange("p (h v) -> p h v", h=2)

                # ---- MLP on plane k
                hp = hpp.tile([128, ZS * D], f32, space="PSUM")
                for h in (0, 1):
                    nc.tensor.matmul(
                        hp[:], w1v[:, h, :], pxv[:, h, :], start=(h == 0), stop=(h == 1)
                    )
                hs = hsp.tile([128, ZS * D], dts)
                nc.scalar.activation(hs[:], hp[:], AF.Relu)
                # y2^T: [c-half partitions, vox] so sigmoid output is c-major
                y2 = y2p.tile([128, VOX2], f32, space="PSUM")
                for h in (0, 1):
                    nc.tensor.matmul(
                        y2[:, h * ZS * D:(h + 1) * ZS * D],
                        w2sb[:, h * 128:(h + 1) * 128],
                        hs[:],
                        start=True,
                        stop=True,
                    )
                sg = sgp.tile([128, VOX2], dts)
                nc.scalar.activation(sg[:], y2[:], AF.Sigmoid)
                sg_t[k] = sg

            # ---- dense multiply with the contiguous center slab, plane
            # k2 = i-5 (sg is 2 planes old -> DVE never waits on ACT)
            k2 = i - 5
            if 0 <= k2 < NX:
                prod = prp.tile([128, VOX2], dts)
                nc.vector.tensor_mul(prod[:], sg_t.pop(k2)[:], P_t.pop(k2))
                if k2 % 2 == 1:
                    Pc_t.pop(k2 // 2)
                nc.gpsimd.dma_start(out[k2 * 128:(k2 + 1) * 128, :], prod[:])

        for t in range(NX // 2):
            # ---- load 2 planes: ladder tile + center-slab tile
            P2 = pp.tile([128, 2 * PLANE], dts, name="P")
            nc.sync.dma_start(
                P2[:].rearrange("p (b f) -> p b f", b=2),
                g[t * 256:(t + 1) * 256, :].rearrange("(b p) f -> p b f", b=2),
            )
            cen2 = cenp.tile([128, 2 * VOX2], dts, tag="cen")
            nc.scalar.dma_start(
                cen2[:].rearrange("p (b f) -> p b f", b=2),
                gc[t * 256:(t + 1) * 256, :].rearrange("(b p) f -> p b f", b=2),
            )
            Pc_t[t] = cen2
            P_t[2 * t] = cen2[:, 0:VOX2]
            P_t[2 * t + 1] = cen2[:, VOX2:2 * VOX2]

            # ---- z-ladder, both planes per instruction (flat runs per (x,h))
            Pg = P2[:].rearrange("p (g f) -> p g f", g=4)
            m2z = ztp.tile([128, 4 * (ZH - 1) * D], dts, tag="m2z")
            m2zg = m2z[:].rearrange("p (g f) -> p g f", g=4)
            nc.vector.tensor_max(
                m2zg, Pg[:, :, 0:(ZH - 1) * D], Pg[:, :, D:ZH * D])
            m4z = ztp.tile([128, 4 * (ZH - 3) * D], dts, tag="m4z")
            m4zg = m4z[:].rearrange("p (g f) -> p g f", g=4)
            nc.vector.tensor_max(
                m4zg, m2zg[:, :, 0:(ZH - 3) * D],
                m2zg[:, :, 2 * D:(ZH - 1) * D])
            m4zv = m4z[:].rearrange("p (g z y) -> p g z y", g=4, z=ZH - 3)
            nc.vector.tensor_max(
                ypv[:, :, :, 4:4 + D], m4zv[:, :, 0:ZS, :], m4zv[:, :, 3:3 + ZS, :])

            # ---- y-ladder, both planes per instruction ((x,h,z) merged rows)
            m2y = ytp.tile([128, 4 * ZS * (YP - 2)], dts, tag="m2y")
            m2ym = m2y[:].rearrange("p (g y) -> p g y", g=4 * ZS)
            nc.vector.tensor_max(
                m2ym, ypg[:, :, 1:YP - 1], ypg[:, :, 2:YP])
            m4y = ytp.tile([128, 4 * ZS * (YP - 4)], dts, tag="m4y")
            m4ym = m4y[:].rearrange("p (g y) -> p g y", g=4 * ZS)
            nc.vector.tensor_max(
                m4ym, m2ym[:, :, 0:YP - 4], m2ym[:, :, 2:YP - 2])
            oy2 = oyp.tile([128, 2 * VOX2], dts, tag="oy")
            oy2m = oy2[:].rearrange("p (g y) -> p g y", g=4 * ZS)
            nc.vector.tensor_max(
                oy2m, m4ym[:, :, 0:D], m4ym[:, :, 3:3 + D])
            oy_t[2 * t] = oy2[:, 0:VOX2]
            oy_t[2 * t + 1] = oy2[:, VOX2:2 * VOX2]

            plane_tail(2 * t)
            plane_tail(2 * t + 1)

        for i in range(NX, NX + 5):
            oy_t[i] = neg[:]
            plane_tail(i)

    nc.compile()
    return nc


def host_prep(cfg: Cfg, feats, coords, W1, W2):
    """Build per-core dense grids in device layout. Returns in_maps."""
    import ml_dtypes

    bf16 = ml_dtypes.bfloat16
    D, ZS, ZH, NX = cfg.D, cfg.ZS, cfg.ZH, cfg.NX

    ix = coords[:, 0].astype(np.int64)
    iy = coords[:, 1].astype(np.int64)
    iz = coords[:, 2].astype(np.int64)

    fb = feats.astype(bf16)
    # rows in [p, h] order: channel c lives at partition c%128, half c//128
    rows = np.ascontiguousarray(fb.reshape(-1, 2, 128).transpose(0, 2, 1))
    w1h = np.ascontiguousarray(W1.astype(bf16))
    w2h = np.ascontiguousarray(W2.astype(bf16))

    in_maps = []
    for k in range(cfg.ncores):
        zlo = k * ZS - 3
        sel = (iz >= zlo) & (iz < zlo + ZH)
        G = np.full((NX, 128, 2, ZH, D), SENT, bf16)
        G[ix[sel], :, :, iz[sel] - zlo, iy[sel]] = rows[sel]
        gc = np.ascontiguousarray(G[:, :, :, 3:3 + ZS, :])
        in_maps.append({
            "g": G.reshape(NX * 128, cfg.PLANE),
            "gc": gc.reshape(NX * 128, cfg.VOX2),
            "w1": w1h,
            "w2": w2h,
        })
    return in_maps


def host_gather(cfg: Cfg, results, coords):
    """Gather sparse rows from the dense per-core product slabs."""
    D, ZS, ZH, NX = cfg.D, cfg.ZS, cfg.ZH, cfg.NX
    ix = coords[:, 0].astype(np.int64)
    iy = coords[:, 1].astype(np.int64)
    iz = coords[:, 2].astype(np.int64)
    out_full = np.empty((coords.shape[0], C), np.float32)
    for k in range(cfg.ncores):
        own = (iz >= k * ZS) & (iz < (k + 1) * ZS)
        O = results[k]["out"].reshape(NX, 128, 2, ZS, D)
        vals = O[ix[own], :, :, iz[own] - k * ZS, iy[own]]  # [n, 128, 2]
        out_full[own] = (
            vals.transpose(0, 2, 1).reshape(-1, C).astype(np.float32)
        )
    return out_full


_CACHE = {}


def _get_nc(cfg: Cfg):
    if cfg not in _CACHE:
        _CACHE[cfg] = build_nc(cfg)
    return _CACHE[cfg]


def kernel(feats, coords, W1, W2):
    from concourse.bass_utils import run_bass_kernel_spmd

    cfg = FULL
    nc = _get_nc(cfg)
    coords = np.asarray(coords)
    in_maps = host_prep(
        cfg,
        np.asarray(feats, np.float32),
        coords,
        np.asarray(W1, np.float32),
        np.asarray(W2, np.float32),
    )
    res = run_bass_kernel_spmd(nc, in_maps, core_ids=list(range(cfg.ncores)))
    return host_gather(cfg, res.results, coords)
